# revision 19
# baseline (speedup 1.0000x reference)
"""nn_Cate3Classifier — 8-core Trainium2 Bass kernel (fp8 DoubleRow).

Math (see reference):
    h   = swem_vec @ W_fc (+ b_fc)        # b_fc cancels inside BatchNorm
    hn  = relu((h - mu) * rsqrt(var + eps) * gamma + beta)   # batch stats over ALL B rows
    out = hn @ W_clf + b_clf
    out[i, j] = -100 where mask2[cate2[i], j]

Distribution: pure data-parallel over the batch (2048 rows/core on 8 cores);
weights/mask table replicated. BN batch statistics: ONE ncfw AllGather of
per-core [sum_h, sum_h^2] (bf16 [128,8]) + local tree-fold.

Design notes (trace-driven):
  - fc matmul in fp8 e4m3 DoubleRow perf mode (2 contraction rows/cycle).
    W_fc pre-scaled by 64 host-side (e4m3 range); the scale cancels in BN
    (eps scaled by 64^2).
  - x marshaled [KP, 2, 128, BL] so every load is a fully-linear
    2048B-per-partition DMA; 24 transfers rotate over the sync/scalar/gpsimd
    queues kp-ascending. The feed is HBM/descriptor-supply bound (~150GB/s),
    which together with the PE floor puts fc at ~33-35us.
  - fc runs f-grouped passes (f01 with 8 PSUM banks, then f2, f3) so each
    stationary weight pair serves 2048 rhs cols (half the LDWEIGHTS) and
    only f3's four drains gate the stats fold.
  - EXACTLY ONE collective. Measured: the ncfw entry barrier grows ~13us per
    additional cc op (a warmup AllGather is strictly counterproductive), cc
    triggers are gated on the entry barrier, and the first op pays ~11.5us
    cold dispatch. Keep nothing queued behind the trigger on gpsimd.
  - ACT tables for Sqrt/Relu preloaded via dummy activations in the post-fc
    idle window (the lazy table swap was 1.28us on the critical path).
  - BN consts all-vector except the Sqrt (constants folded into stt
    immediates); BN apply on ACT (per-partition scale/bias native) in
    256/256/512/1024-col groups: first clf matmuls start ~1.8us after
    consts while later groups amortize ACT op overhead.
  - clf bias added on DVE in-PSUM (tensor_tensor with a host-replicated
    [128,125] bias tile) instead of 32 extra PE instructions; masking via
    copy_predicated into a -100-memset buffer using an indirect-DMA
    gathered keep-table.
  - output stored partition-major as 4 fully-linear 2KB/partition DMAs
    alternating sync/scalar; host unshuffles.
"""

import numpy as np
import ml_dtypes

B, D, H, C3, C2 = 16384, 2048, 512, 125, 64
NCORES = 8
BL = B // NCORES      # 2048 rows per core
KP = D // 256         # 8 fp8 DoubleRow contraction pairs (2x128 each)
RF = H // 128         # 4 feature chunks
NRC = BL // 512       # 4 row chunks of 512
NT = BL // 128        # 16 output row tiles of 128
BN_EPS = 1e-5
MASK_VAL = -100.0
WSCALE = 64.0         # host pre-scale on W_fc; cancels in BN (eps scaled too)

_CACHE = {}


def _build_nc():
    from contextlib import ExitStack

    import concourse.bass as bass
    import concourse.tile as tile
    from concourse import bacc, mybir

    f32 = mybir.dt.float32
    bf16 = mybir.dt.bfloat16
    fp8 = mybir.dt.float8e4
    i32 = mybir.dt.int32
    AF = mybir.ActivationFunctionType
    OP = mybir.AluOpType
    DR = mybir.MatmulPerfMode.DoubleRow

    nc = bacc.Bacc("TRN2", target_bir_lowering=False, debug=False, num_devices=NCORES)

    xq_d = nc.dram_tensor("xq", [KP, 2, 128, BL], fp8, kind="ExternalInput")
    wfc_d = nc.dram_tensor("wfc", [128, KP * RF * 2 * 128], fp8, kind="ExternalInput")
    wclf_d = nc.dram_tensor("wclf", [128, RF * C3], bf16, kind="ExternalInput")
    bclf_d = nc.dram_tensor("bclf", [128, C3], f32, kind="ExternalInput")
    m2_d = nc.dram_tensor("m2", [C2, C3], mybir.dt.uint8, kind="ExternalInput")
    cat_d = nc.dram_tensor("cat", [128, NT], i32, kind="ExternalInput")
    # partition-major output: out[p, t*C3+c] = row t*128+p; host unshuffles.
    out_d = nc.dram_tensor("out", [128, NT * C3], f32, kind="ExternalOutput")

    with tile.TileContext(nc) as tc, ExitStack() as ctx:
        xpool = ctx.enter_context(tc.tile_pool(name="xq", bufs=KP))
        wpool = ctx.enter_context(tc.tile_pool(name="w", bufs=1))
        hpool = ctx.enter_context(tc.tile_pool(name="h", bufs=RF))
        hnpool = ctx.enter_context(tc.tile_pool(name="hn", bufs=RF))
        hsqpool = ctx.enter_context(tc.tile_pool(name="hsq", bufs=2))
        smallpool = ctx.enter_context(tc.tile_pool(name="small", bufs=1))
        psum_fc = ctx.enter_context(tc.tile_pool(name="psfc", bufs=8, space="PSUM"))
        drampool = ctx.enter_context(tc.tile_pool(name="dram", bufs=1, space="DRAM"))

        # ---- loads: fully-linear DMAs rotated over 3 queues, kp-ascending;
        # small late-use tensors (cate/bclf/wclf) load after the x stream.
        wfc_sb = wpool.tile([128, KP * RF * 2 * 128], fp8, tag="wfc")
        xts = [xpool.tile([128, 2 * BL], fp8, tag="xq", name=f"xq{k}") for k in range(KP)]

        qs = [nc.sync, nc.scalar, nc.gpsimd]
        WCH = RF * 2 * 128  # 1024 wfc cols per pair
        for kp in range(KP):
            qs[kp % 3].dma_start(
                wfc_sb[:, kp * WCH : (kp + 1) * WCH],
                wfc_d.ap()[:, kp * WCH : (kp + 1) * WCH],
            )
            for j in range(2):
                qs[(kp + 1 + j) % 3].dma_start(
                    xts[kp][:, j * BL : (j + 1) * BL],
                    xq_d.ap()[kp, j],
                )

        cate_sb = smallpool.tile([128, NT], i32, tag="cate")
        nc.sync.dma_start(cate_sb[:], cat_d.ap())
        bclf_sb = smallpool.tile([128, C3], f32, tag="bclf")
        nc.sync.dma_start(bclf_sb[:], bclf_d.ap())
        wclf_sb = wpool.tile([128, RF * C3], bf16, tag="wclf")
        nc.sync.dma_start(wclf_sb[:], wclf_d.ap())

        def x3(ap):  # [128, 2*BL] -> [128, 2, BL]
            return ap.rearrange("p (j r) -> p j r", j=2)

        eps_sb = smallpool.tile([128, 1], f32, tag="eps")
        nc.vector.memset(eps_sb[:], BN_EPS * WSCALE * WSCALE)

        # ---- fc matmul (fp8 DoubleRow) + streaming BN stats ----
        h_sb = [hpool.tile([128, BL], bf16, tag="h", name=f"h{f}") for f in range(RF)]
        # stats col layout: r*8 + f = sum_h, r*8 + 4 + f = sum_h^2
        sums_sb = smallpool.tile([128, NRC * 8], f32, tag="sums")

        def wslice(kp, f):  # [128, 2, 128] stationary pair
            off = ((kp * RF + f) * 2) * 128
            return wfc_sb[:, off : off + 256].rearrange("p (j m) -> p j m", j=2)

        def drain(f, r, psum):
            nc.vector.tensor_scalar(
                out=h_sb[f][:, r * 512 : (r + 1) * 512],
                in0=psum[:],
                scalar1=1.0,
                scalar2=None,
                op0=OP.mult,
                op1=OP.add,
                accum_out=sums_sb[:, r * 8 + f : r * 8 + f + 1],
            )
            hsq = hsqpool.tile([128, 512], bf16, tag="hsq", name=f"hsq{f}_{r}")
            nc.scalar.activation(
                out=hsq[:],
                in_=psum[:],
                func=AF.Square,
                accum_out=sums_sb[:, r * 8 + 4 + f : r * 8 + 4 + f + 1],
            )

        # pass A: f-pair (0,1), all 4 row chunks per stationary weight pair
        # (2048 rhs cols per LDWEIGHTS); then f=2 and f=3 as separate
        # sub-passes so only f3's four drains gate the stats fold at the end.
        def fc_pass(fs):
            psums = {
                (f, r): psum_fc.tile([128, 512], f32, tag="ps", name=f"ps{f}_{r}")
                for f in fs
                for r in range(4)
            }
            for kp in range(KP):
                for f in fs:
                    lhsT = wslice(kp, f)
                    for r in range(4):
                        nc.tensor.matmul(
                            psums[(f, r)][:],
                            lhsT=lhsT,
                            rhs=x3(xts[kp][:])[:, :, r * 512 : (r + 1) * 512],
                            start=(kp == 0),
                            stop=(kp == KP - 1),
                            perf_mode=DR,
                        )
            for f in fs:
                for r in range(4):
                    drain(f, r, psums[(f, r)])

        fc_pass((0, 1))
        fc_pass((2,))
        fc_pass((3,))

        # ACT table preload: Sqrt/Relu live in a different table than Square;
        # force the swap now (ACT idle, AG in flight) instead of paying
        # ~1.3us on the post-AG critical path. Same queue as the Squares ->
        # runs right after the last drain.
        tbl_sb = smallpool.tile([128, 1], f32, tag="tbl")
        nc.scalar.activation(out=tbl_sb[:], in_=eps_sb[:], func=AF.Sqrt)
        nc.scalar.activation(out=tbl_sb[:], in_=eps_sb[:], func=AF.Relu)

        # fold the row-chunk partials (vector queue, right before the
        # collective trigger below)
        t01 = smallpool.tile([128, 8], f32, tag="t01")
        nc.vector.tensor_tensor(out=t01[:], in0=sums_sb[:, 0:8], in1=sums_sb[:, 8:16], op=OP.add)
        t23 = smallpool.tile([128, 8], f32, tag="t23")
        nc.vector.tensor_tensor(out=t23[:], in0=sums_sb[:, 16:24], in1=sums_sb[:, 24:32], op=OP.add)
        stats = smallpool.tile([128, 8], bf16, tag="stats")
        nc.vector.tensor_tensor(out=stats[:], in0=t01[:], in1=t23[:], op=OP.add)

        # ---- stats AllGather across the 8 cores + local fold ----
        cc_in = drampool.tile([128, 8], bf16, tag="ccin")
        cc_out = drampool.tile([NCORES, 128, 8], bf16, tag="ccout")
        ccdma = nc.sync.dma_start(cc_in[:], stats[:])
        nc.gpsimd.collective_compute(
            "AllGather",
            OP.bypass,
            replica_groups=[list(range(NCORES))],
            ins=[cc_in[:].opt()],
            outs=[cc_out[:].opt()],
        )
        # masked positions default to -100; gathers fill the keep-table
        outs_sb = smallpool.tile([128, NT * C3], f32, tag="outs")
        nc.gpsimd.memset(outs_sb[:], MASK_VAL)
        mask_sb = smallpool.tile([128, NT * C3], mybir.dt.uint8, tag="mask")
        for t in range(NT):
            nc.gpsimd.indirect_dma_start(
                out=mask_sb[:, t * C3 : (t + 1) * C3],
                out_offset=None,
                in_=m2_d.ap(),
                in_offset=bass.IndirectOffsetOnAxis(ap=cate_sb[:, t : t + 1], axis=0),
            )

        gath = smallpool.tile([128, 8 * NCORES], bf16, tag="gath")
        nc.gpsimd.dma_start(
            gath[:].rearrange("p (r c) -> p r c", r=NCORES),
            cc_out[:].rearrange("r p c -> p r c"),
        )
        g1 = smallpool.tile([128, 32], f32, tag="g1")
        nc.vector.tensor_tensor(out=g1[:], in0=gath[:, 0:32], in1=gath[:, 32:64], op=OP.add)
        g2 = smallpool.tile([128, 16], f32, tag="g2")
        nc.vector.tensor_tensor(out=g2[:], in0=g1[:, 0:16], in1=g1[:, 16:32], op=OP.add)
        stats_all = smallpool.tile([128, 8], f32, tag="statsall")
        nc.vector.tensor_tensor(out=stats_all[:], in0=g2[:, 0:8], in1=g2[:, 8:16], op=OP.add)

        # PE warm-up during the AG wait: HAM throttles an idle PE; dummy
        # matmuls (gated on the local stats DMA, NOT the collective) keep
        # duty up for the clf matmuls.
        warm_ps = psum_fc.tile([128, 512], f32, tag="ps", name="warmps")
        for wi in range(16):
            mi = nc.tensor.matmul(
                warm_ps[:],
                lhsT=h_sb[0][:, 0:128],
                rhs=h_sb[0][:, 0:512],
                start=True,
                stop=True,
                skip_group_check=True,
            )
            if wi == 0:
                tile.add_dep_helper(mi.ins, ccdma.ins, sync=True, reason="warm PE during AG")

        # ---- BN consts: svec = rsqrt(var'+eps'), tvec = -mu'*svec ----
        # (scaled domain: stats are of h' = 64*h, eps' = eps*64^2; gamma==1 /
        # beta==0 per setup_inputs — asserted host-side. All-vector except the
        # Sqrt; constants folded into stt immediates.)
        S1 = stats_all[:, 0:RF]
        S2 = stats_all[:, RF : 2 * RF]
        m2s = smallpool.tile([128, RF], f32, tag="m2s")
        nc.vector.scalar_tensor_tensor(
            out=m2s[:], in0=S1, scalar=1.0 / (B * B), in1=S1, op0=OP.mult, op1=OP.mult,
        )
        var = smallpool.tile([128, RF], f32, tag="var")
        nc.vector.scalar_tensor_tensor(
            out=var[:], in0=S2, scalar=1.0 / B, in1=m2s[:], op0=OP.mult, op1=OP.subtract,
        )
        std = smallpool.tile([128, RF], f32, tag="std")
        nc.scalar.activation(std[:], var[:], AF.Sqrt, bias=eps_sb[:, 0:1])
        svec = smallpool.tile([128, RF], f32, tag="svec")
        nc.vector.reciprocal(svec[:], std[:])
        tvec = smallpool.tile([128, RF], f32, tag="tvec")
        nc.vector.scalar_tensor_tensor(
            out=tvec[:], in0=S1, scalar=-1.0 / B, in1=svec[:], op0=OP.mult, op1=OP.mult,
        )

        # ---- BN apply + relu, then clf matmul + bias + mask + store ----
        # BN apply on ACT (per-partition scale/bias native there) in 512-col
        # chunks: the first 4-tile clf group starts ~2.6us after consts.
        # Bias is added on DVE (broadcast tensor_tensor) — saves 32 PE
        # instructions vs the bias-matmul trick.
        hn_sb = [hnpool.tile([128, BL], bf16, tag="hn", name=f"hn{f}") for f in range(RF)]
        groups = [(0, 256), (256, 512), (512, 1024), (1024, 2048)]
        for gi, (c0, c1) in enumerate(groups):
            for f in range(RF):
                nc.scalar.activation(
                    out=hn_sb[f][:, c0:c1],
                    in_=h_sb[f][:, c0:c1],
                    func=AF.Relu,
                    scale=svec[:, f : f + 1],
                    bias=tvec[:, f : f + 1],
                )
            for t in range(c0 // 128, c1 // 128):
                po = psum_fc.tile([128, C3], f32, tag="ps", name=f"po{t}")
                for f in range(RF):
                    nc.tensor.matmul(
                        po[:],
                        lhsT=hn_sb[f][:, t * 128 : (t + 1) * 128],
                        rhs=wclf_sb[:, f * C3 : (f + 1) * C3],
                        start=(f == 0),
                        stop=(f == RF - 1),
                    )
                nc.vector.tensor_tensor(out=po[:], in0=po[:], in1=bclf_sb[:], op=OP.add)
                nc.vector.copy_predicated(
                    outs_sb[:, t * C3 : (t + 1) * C3],
                    mask_sb[:, t * C3 : (t + 1) * C3],
                    po[:],
                )
                # one fully-linear store per 4-tile group (2KB/partition)
                if t % 4 == 3:
                    t0 = t - 3
                    gs = slice(t0 * C3, (t0 + 4) * C3)
                    eng = nc.sync if (t0 // 4) % 2 == 0 else nc.scalar
                    eng.dma_start(out_d.ap()[:, gs], outs_sb[:, gs])

    nc.compile()
    return nc


def _get_nc():
    if "nc" not in _CACHE:
        _CACHE["nc"] = _build_nc()
    return _CACHE["nc"]


def make_in_maps(**inputs):
    """Host-side marshaling: shard/cast/layout the full inputs per core."""
    bf16 = ml_dtypes.bfloat16
    e4m3 = ml_dtypes.float8_e4m3  # IEEE variant, max 240 — matches TRN FP8_EXP4

    x = np.asarray(inputs["swem_vec"], dtype=np.float32)
    xT8 = np.ascontiguousarray(x.T).astype(e4m3)  # [D, B]

    wfc = np.asarray(inputs["W_fc"], dtype=np.float32) * WSCALE
    wfc8 = np.ascontiguousarray(
        wfc.reshape(KP, 2, 128, RF, 128).transpose(2, 0, 3, 1, 4).reshape(128, -1)
    ).astype(e4m3)

    wclf = np.asarray(inputs["W_clf"], dtype=np.float32)
    wclf_h = np.ascontiguousarray(
        wclf.reshape(RF, 128, C3).transpose(1, 0, 2).reshape(128, RF * C3)
    ).astype(bf16)
    bclf = np.ascontiguousarray(np.broadcast_to(np.asarray(inputs["b_clf"], dtype=np.float32)[None, :], (128, C3)))
    # the device program specializes gamma==1 / beta==0 (reference
    # setup_inputs hardcodes them); fail loudly if that ever changes
    assert np.all(np.asarray(inputs["gamma"], dtype=np.float32) == 1.0), "gamma != 1"
    assert np.all(np.asarray(inputs["beta"], dtype=np.float32) == 0.0), "beta != 0"
    m2 = (~np.asarray(inputs["mask2"])).astype(np.uint8)  # 1 = keep, 0 = mask to -100
    cate = np.asarray(inputs["cate2"]).astype(np.int32)

    in_maps = []
    for c in range(NCORES):
        sl = slice(c * BL, (c + 1) * BL)
        xc = xT8[:, sl]  # [D, BL]
        # [KP, 2, 128, BL]: d = kp*256 + j*128 + p — each (kp, j) block is a
        # fully-linear 256KB DMA
        xq = np.ascontiguousarray(xc.reshape(KP, 2, 128, BL))
        in_maps.append(
            {
                "xq": xq,
                "wfc": wfc8,
                "wclf": wclf_h,
                "bclf": bclf,
                "m2": m2,
                "cat": np.ascontiguousarray(cate[sl].reshape(NT, 128).T),
            }
        )
    return in_maps


def run(in_maps, trace=False, **kwargs):
    from concourse.bass_utils import run_bass_kernel_spmd

    nc = _get_nc()
    return run_bass_kernel_spmd(
        nc, in_maps, core_ids=list(range(NCORES)), trace=trace, **kwargs
    )


def unshard(res) -> np.ndarray:
    # device output is partition-major [128, NT*C3]; unshuffle to [BL, C3]
    return np.concatenate(
        [
            res.results[c]["out"].reshape(128, NT, C3).transpose(1, 0, 2).reshape(BL, C3)
            for c in range(NCORES)
        ],
        axis=0,
    )


def kernel(**inputs) -> np.ndarray:
    in_maps = make_in_maps(**inputs)
    return unshard(run(in_maps, trace=False))


# revision 20
# speedup vs baseline: 1.6212x; 1.6212x over previous
"""nn_Cate3Classifier — 8-core Trainium2 Bass kernel (fp8 DoubleRow).

Math (see reference):
    h   = swem_vec @ W_fc (+ b_fc)        # b_fc cancels inside BatchNorm
    hn  = relu((h - mu) * rsqrt(var + eps) * gamma + beta)   # batch stats over ALL B rows
    out = hn @ W_clf + b_clf
    out[i, j] = -100 where mask2[cate2[i], j]

Distribution: pure data-parallel over the batch (2048 rows/core on 8 cores);
weights/mask table replicated. BN batch statistics: ONE ncfw AllGather of
per-core [sum_h, sum_h^2] (bf16 [128,8]) + local tree-fold.

Design notes (trace-driven):
  - fc matmul in fp8 e4m3 DoubleRow perf mode (2 contraction rows/cycle).
    W_fc pre-scaled by 64 host-side (e4m3 range); the scale cancels in BN
    (eps scaled by 64^2).
  - x marshaled [KP, 2, 128, BL] so every load is a fully-linear
    2048B-per-partition DMA; 24 transfers rotate over the sync/scalar/gpsimd
    queues kp-ascending. The feed is HBM/descriptor-supply bound (~150GB/s),
    which together with the PE floor puts fc at ~33-35us.
  - fc runs f-grouped passes (f01 with 8 PSUM banks, then f2, f3) so each
    stationary weight pair serves 2048 rhs cols (half the LDWEIGHTS) and
    only f3's four drains gate the stats fold.
  - EXACTLY ONE collective. Measured: the ncfw entry barrier grows ~13us per
    additional cc op (a warmup AllGather is strictly counterproductive), cc
    triggers are gated on the entry barrier, and the first op pays ~11.5us
    cold dispatch. Keep nothing queued behind the trigger on gpsimd.
  - ACT tables for Sqrt/Relu preloaded via dummy activations in the post-fc
    idle window (the lazy table swap was 1.28us on the critical path).
  - BN consts all-vector except the Sqrt (constants folded into stt
    immediates); BN apply on ACT (per-partition scale/bias native) in
    256/256/512/1024-col groups: first clf matmuls start ~1.8us after
    consts while later groups amortize ACT op overhead.
  - clf bias added on DVE in-PSUM (tensor_tensor with a host-replicated
    [128,125] bias tile) instead of 32 extra PE instructions; masking via
    copy_predicated into a -100-memset buffer using an indirect-DMA
    gathered keep-table.
  - output stored partition-major as 4 fully-linear 2KB/partition DMAs
    alternating sync/scalar; host unshuffles.
"""

import numpy as np
import ml_dtypes

B, D, H, C3, C2 = 16384, 2048, 512, 125, 64
NCORES = 8
BL = B // NCORES      # 2048 rows per core
KP = D // 256         # 8 fp8 DoubleRow contraction pairs (2x128 each)
RF = H // 128         # 4 feature chunks
NRC = BL // 512       # 4 row chunks of 512
NT = BL // 128        # 16 output row tiles of 128
BN_EPS = 1e-5
MASK_VAL = -100.0
WSCALE = 64.0         # host pre-scale on W_fc; cancels in BN (eps scaled too)

_CACHE = {}


def _build_nc():
    from contextlib import ExitStack

    import concourse.bass as bass
    import concourse.tile as tile
    from concourse import bacc, mybir

    f32 = mybir.dt.float32
    bf16 = mybir.dt.bfloat16
    fp8 = mybir.dt.float8e4
    i32 = mybir.dt.int32
    AF = mybir.ActivationFunctionType
    OP = mybir.AluOpType
    DR = mybir.MatmulPerfMode.DoubleRow

    nc = bacc.Bacc("TRN2", target_bir_lowering=False, debug=False, num_devices=NCORES)

    xq_d = nc.dram_tensor("xq", [KP, 2, 128, BL], fp8, kind="ExternalInput")
    wfc_d = nc.dram_tensor("wfc", [128, KP * RF * 2 * 128], fp8, kind="ExternalInput")
    wclf_d = nc.dram_tensor("wclf", [128, RF * C3], bf16, kind="ExternalInput")
    bclf_d = nc.dram_tensor("bclf", [128, C3], f32, kind="ExternalInput")
    m2_d = nc.dram_tensor("m2", [C2, C3], mybir.dt.uint8, kind="ExternalInput")
    cat_d = nc.dram_tensor("cat", [128, NT], i32, kind="ExternalInput")
    # partition-major output: out[p, t*C3+c] = row t*128+p; host unshuffles.
    out_d = nc.dram_tensor("out", [128, NT * C3], f32, kind="ExternalOutput")

    with tile.TileContext(nc) as tc, ExitStack() as ctx:
        xpool = ctx.enter_context(tc.tile_pool(name="xq", bufs=KP))
        wpool = ctx.enter_context(tc.tile_pool(name="w", bufs=1))
        hpool = ctx.enter_context(tc.tile_pool(name="h", bufs=RF))
        hnpool = ctx.enter_context(tc.tile_pool(name="hn", bufs=RF))
        hsqpool = ctx.enter_context(tc.tile_pool(name="hsq", bufs=2))
        smallpool = ctx.enter_context(tc.tile_pool(name="small", bufs=1))
        psum_fc = ctx.enter_context(tc.tile_pool(name="psfc", bufs=8, space="PSUM"))
        drampool = ctx.enter_context(tc.tile_pool(name="dram", bufs=1, space="DRAM"))

        # ---- loads: fully-linear DMAs rotated over 3 queues, kp-ascending;
        # small late-use tensors (cate/bclf/wclf) load after the x stream.
        wfc_sb = wpool.tile([128, KP * RF * 2 * 128], fp8, tag="wfc")
        xts = [xpool.tile([128, 2 * BL], fp8, tag="xq", name=f"xq{k}") for k in range(KP)]

        qs = [nc.sync, nc.scalar, nc.gpsimd]
        WCH = RF * 2 * 128  # 1024 wfc cols per pair
        for kp in range(KP):
            qs[kp % 3].dma_start(
                wfc_sb[:, kp * WCH : (kp + 1) * WCH],
                wfc_d.ap()[:, kp * WCH : (kp + 1) * WCH],
            )
            for j in range(2):
                qs[(kp + 1 + j) % 3].dma_start(
                    xts[kp][:, j * BL : (j + 1) * BL],
                    xq_d.ap()[kp, j],
                )

        cate_sb = smallpool.tile([128, NT], i32, tag="cate")
        nc.sync.dma_start(cate_sb[:], cat_d.ap())
        bclf_sb = smallpool.tile([128, C3], f32, tag="bclf")
        nc.sync.dma_start(bclf_sb[:], bclf_d.ap())
        wclf_sb = wpool.tile([128, RF * C3], bf16, tag="wclf")
        nc.sync.dma_start(wclf_sb[:], wclf_d.ap())

        def x3(ap):  # [128, 2*BL] -> [128, 2, BL]
            return ap.rearrange("p (j r) -> p j r", j=2)

        eps_sb = smallpool.tile([128, 1], f32, tag="eps")
        nc.vector.memset(eps_sb[:], BN_EPS * WSCALE * WSCALE)

        # ---- fc matmul (fp8 DoubleRow) + streaming BN stats ----
        h_sb = [hpool.tile([128, BL], bf16, tag="h", name=f"h{f}") for f in range(RF)]
        # stats col layout: r*8 + f = sum_h, r*8 + 4 + f = sum_h^2
        sums_sb = smallpool.tile([128, NRC * 8], f32, tag="sums")

        def wslice(kp, f):  # [128, 2, 128] stationary pair
            off = ((kp * RF + f) * 2) * 128
            return wfc_sb[:, off : off + 256].rearrange("p (j m) -> p j m", j=2)

        def drain(f, r, psum):
            nc.vector.tensor_scalar(
                out=h_sb[f][:, r * 512 : (r + 1) * 512],
                in0=psum[:],
                scalar1=1.0,
                scalar2=None,
                op0=OP.mult,
                op1=OP.add,
                accum_out=sums_sb[:, r * 8 + f : r * 8 + f + 1],
            )
            hsq = hsqpool.tile([128, 512], bf16, tag="hsq", name=f"hsq{f}_{r}")
            nc.scalar.activation(
                out=hsq[:],
                in_=psum[:],
                func=AF.Square,
                accum_out=sums_sb[:, r * 8 + 4 + f : r * 8 + 4 + f + 1],
            )

        # pass A: f-pair (0,1), all 4 row chunks per stationary weight pair
        # (2048 rhs cols per LDWEIGHTS); then f=2 and f=3 as separate
        # sub-passes so only f3's four drains gate the stats fold at the end.
        def fc_pass(fs):
            psums = {
                (f, r): psum_fc.tile([128, 512], f32, tag="ps", name=f"ps{f}_{r}")
                for f in fs
                for r in range(4)
            }
            for kp in range(KP):
                for f in fs:
                    lhsT = wslice(kp, f)
                    for r in range(4):
                        nc.tensor.matmul(
                            psums[(f, r)][:],
                            lhsT=lhsT,
                            rhs=x3(xts[kp][:])[:, :, r * 512 : (r + 1) * 512],
                            start=(kp == 0),
                            stop=(kp == KP - 1),
                            perf_mode=DR,
                        )
            for f in fs:
                for r in range(4):
                    drain(f, r, psums[(f, r)])

        fc_pass((0, 1))
        fc_pass((2,))
        fc_pass((3,))

        # ACT table preload: Sqrt/Relu live in a different table than Square;
        # force the swap now (ACT idle, AG in flight) instead of paying
        # ~1.3us on the post-AG critical path. Same queue as the Squares ->
        # runs right after the last drain.
        tbl_sb = smallpool.tile([128, 1], f32, tag="tbl")
        nc.scalar.activation(out=tbl_sb[:], in_=eps_sb[:], func=AF.Sqrt)
        nc.scalar.activation(out=tbl_sb[:], in_=eps_sb[:], func=AF.Relu)

        # fold the row-chunk partials (vector queue, right before the
        # collective trigger below)
        t01 = smallpool.tile([128, 8], f32, tag="t01")
        nc.vector.tensor_tensor(out=t01[:], in0=sums_sb[:, 0:8], in1=sums_sb[:, 8:16], op=OP.add)
        t23 = smallpool.tile([128, 8], f32, tag="t23")
        nc.vector.tensor_tensor(out=t23[:], in0=sums_sb[:, 16:24], in1=sums_sb[:, 24:32], op=OP.add)
        stats = smallpool.tile([128, 8], bf16, tag="stats")
        nc.vector.tensor_tensor(out=stats[:], in0=t01[:], in1=t23[:], op=OP.add)

        # ---- stats AllGather across the 8 cores + local fold ----
        cc_in = drampool.tile([128, 8], bf16, tag="ccin")
        cc_out = drampool.tile([NCORES, 128, 8], bf16, tag="ccout", addr_space="Shared")
        ccdma = nc.sync.dma_start(cc_in[:], stats[:])
        nc.gpsimd.collective_compute(
            "AllGather",
            OP.bypass,
            replica_groups=[list(range(NCORES))],
            ins=[cc_in[:].opt()],
            outs=[cc_out[:].opt()],
        )
        # masked positions default to -100; gathers fill the keep-table
        outs_sb = smallpool.tile([128, NT * C3], f32, tag="outs")
        nc.gpsimd.memset(outs_sb[:], MASK_VAL)
        mask_sb = smallpool.tile([128, NT * C3], mybir.dt.uint8, tag="mask")
        for t in range(NT):
            nc.gpsimd.indirect_dma_start(
                out=mask_sb[:, t * C3 : (t + 1) * C3],
                out_offset=None,
                in_=m2_d.ap(),
                in_offset=bass.IndirectOffsetOnAxis(ap=cate_sb[:, t : t + 1], axis=0),
            )

        gath = smallpool.tile([128, 8 * NCORES], bf16, tag="gath")
        nc.gpsimd.dma_start(
            gath[:].rearrange("p (r c) -> p r c", r=NCORES),
            cc_out[:].rearrange("r p c -> p r c"),
        )
        g1 = smallpool.tile([128, 32], f32, tag="g1")
        nc.vector.tensor_tensor(out=g1[:], in0=gath[:, 0:32], in1=gath[:, 32:64], op=OP.add)
        g2 = smallpool.tile([128, 16], f32, tag="g2")
        nc.vector.tensor_tensor(out=g2[:], in0=g1[:, 0:16], in1=g1[:, 16:32], op=OP.add)
        stats_all = smallpool.tile([128, 8], f32, tag="statsall")
        nc.vector.tensor_tensor(out=stats_all[:], in0=g2[:, 0:8], in1=g2[:, 8:16], op=OP.add)

        # PE warm-up during the AG wait: HAM throttles an idle PE; dummy
        # matmuls (gated on the local stats DMA, NOT the collective) keep
        # duty up for the clf matmuls.
        warm_ps = psum_fc.tile([128, 512], f32, tag="ps", name="warmps")
        for wi in range(16):
            mi = nc.tensor.matmul(
                warm_ps[:],
                lhsT=h_sb[0][:, 0:128],
                rhs=h_sb[0][:, 0:512],
                start=True,
                stop=True,
                skip_group_check=True,
            )
            if wi == 0:
                tile.add_dep_helper(mi.ins, ccdma.ins, sync=True, reason="warm PE during AG")

        # ---- BN consts: svec = rsqrt(var'+eps'), tvec = -mu'*svec ----
        # (scaled domain: stats are of h' = 64*h, eps' = eps*64^2; gamma==1 /
        # beta==0 per setup_inputs — asserted host-side. All-vector except the
        # Sqrt; constants folded into stt immediates.)
        S1 = stats_all[:, 0:RF]
        S2 = stats_all[:, RF : 2 * RF]
        m2s = smallpool.tile([128, RF], f32, tag="m2s")
        nc.vector.scalar_tensor_tensor(
            out=m2s[:], in0=S1, scalar=1.0 / (B * B), in1=S1, op0=OP.mult, op1=OP.mult,
        )
        var = smallpool.tile([128, RF], f32, tag="var")
        nc.vector.scalar_tensor_tensor(
            out=var[:], in0=S2, scalar=1.0 / B, in1=m2s[:], op0=OP.mult, op1=OP.subtract,
        )
        std = smallpool.tile([128, RF], f32, tag="std")
        nc.scalar.activation(std[:], var[:], AF.Sqrt, bias=eps_sb[:, 0:1])
        svec = smallpool.tile([128, RF], f32, tag="svec")
        nc.vector.reciprocal(svec[:], std[:])
        tvec = smallpool.tile([128, RF], f32, tag="tvec")
        nc.vector.scalar_tensor_tensor(
            out=tvec[:], in0=S1, scalar=-1.0 / B, in1=svec[:], op0=OP.mult, op1=OP.mult,
        )

        # ---- BN apply + relu, then clf matmul + bias + mask + store ----
        # BN apply on ACT (per-partition scale/bias native there) in 512-col
        # chunks: the first 4-tile clf group starts ~2.6us after consts.
        # Bias is added on DVE (broadcast tensor_tensor) — saves 32 PE
        # instructions vs the bias-matmul trick.
        hn_sb = [hnpool.tile([128, BL], bf16, tag="hn", name=f"hn{f}") for f in range(RF)]
        groups = [(0, 256), (256, 512), (512, 1024), (1024, 2048)]
        for gi, (c0, c1) in enumerate(groups):
            for f in range(RF):
                nc.scalar.activation(
                    out=hn_sb[f][:, c0:c1],
                    in_=h_sb[f][:, c0:c1],
                    func=AF.Relu,
                    scale=svec[:, f : f + 1],
                    bias=tvec[:, f : f + 1],
                )
            for t in range(c0 // 128, c1 // 128):
                po = psum_fc.tile([128, C3], f32, tag="ps", name=f"po{t}")
                for f in range(RF):
                    nc.tensor.matmul(
                        po[:],
                        lhsT=hn_sb[f][:, t * 128 : (t + 1) * 128],
                        rhs=wclf_sb[:, f * C3 : (f + 1) * C3],
                        start=(f == 0),
                        stop=(f == RF - 1),
                    )
                nc.vector.tensor_tensor(out=po[:], in0=po[:], in1=bclf_sb[:], op=OP.add)
                nc.vector.copy_predicated(
                    outs_sb[:, t * C3 : (t + 1) * C3],
                    mask_sb[:, t * C3 : (t + 1) * C3],
                    po[:],
                )
                # one fully-linear store per 4-tile group (2KB/partition)
                if t % 4 == 3:
                    t0 = t - 3
                    gs = slice(t0 * C3, (t0 + 4) * C3)
                    eng = nc.sync if (t0 // 4) % 2 == 0 else nc.scalar
                    eng.dma_start(out_d.ap()[:, gs], outs_sb[:, gs])

    nc.compile()
    return nc


def _get_nc():
    if "nc" not in _CACHE:
        _CACHE["nc"] = _build_nc()
    return _CACHE["nc"]


def make_in_maps(**inputs):
    """Host-side marshaling: shard/cast/layout the full inputs per core."""
    bf16 = ml_dtypes.bfloat16
    e4m3 = ml_dtypes.float8_e4m3  # IEEE variant, max 240 — matches TRN FP8_EXP4

    x = np.asarray(inputs["swem_vec"], dtype=np.float32)
    xT8 = np.ascontiguousarray(x.T).astype(e4m3)  # [D, B]

    wfc = np.asarray(inputs["W_fc"], dtype=np.float32) * WSCALE
    wfc8 = np.ascontiguousarray(
        wfc.reshape(KP, 2, 128, RF, 128).transpose(2, 0, 3, 1, 4).reshape(128, -1)
    ).astype(e4m3)

    wclf = np.asarray(inputs["W_clf"], dtype=np.float32)
    wclf_h = np.ascontiguousarray(
        wclf.reshape(RF, 128, C3).transpose(1, 0, 2).reshape(128, RF * C3)
    ).astype(bf16)
    bclf = np.ascontiguousarray(np.broadcast_to(np.asarray(inputs["b_clf"], dtype=np.float32)[None, :], (128, C3)))
    # the device program specializes gamma==1 / beta==0 (reference
    # setup_inputs hardcodes them); fail loudly if that ever changes
    assert np.all(np.asarray(inputs["gamma"], dtype=np.float32) == 1.0), "gamma != 1"
    assert np.all(np.asarray(inputs["beta"], dtype=np.float32) == 0.0), "beta != 0"
    m2 = (~np.asarray(inputs["mask2"])).astype(np.uint8)  # 1 = keep, 0 = mask to -100
    cate = np.asarray(inputs["cate2"]).astype(np.int32)

    in_maps = []
    for c in range(NCORES):
        sl = slice(c * BL, (c + 1) * BL)
        xc = xT8[:, sl]  # [D, BL]
        # [KP, 2, 128, BL]: d = kp*256 + j*128 + p — each (kp, j) block is a
        # fully-linear 256KB DMA
        xq = np.ascontiguousarray(xc.reshape(KP, 2, 128, BL))
        in_maps.append(
            {
                "xq": xq,
                "wfc": wfc8,
                "wclf": wclf_h,
                "bclf": bclf,
                "m2": m2,
                "cat": np.ascontiguousarray(cate[sl].reshape(NT, 128).T),
            }
        )
    return in_maps


def run(in_maps, trace=False, **kwargs):
    from concourse.bass_utils import run_bass_kernel_spmd

    nc = _get_nc()
    return run_bass_kernel_spmd(
        nc, in_maps, core_ids=list(range(NCORES)), trace=trace, **kwargs
    )


def unshard(res) -> np.ndarray:
    # device output is partition-major [128, NT*C3]; unshuffle to [BL, C3]
    return np.concatenate(
        [
            res.results[c]["out"].reshape(128, NT, C3).transpose(1, 0, 2).reshape(BL, C3)
            for c in range(NCORES)
        ],
        axis=0,
    )


def kernel(**inputs) -> np.ndarray:
    in_maps = make_in_maps(**inputs)
    return unshard(run(in_maps, trace=False))


# revision 21
# speedup vs baseline: 1.6483x; 1.0167x over previous
"""nn_Cate3Classifier — 8-core Trainium2 Bass kernel (fp8 DoubleRow).

Math (see reference):
    h   = swem_vec @ W_fc (+ b_fc)        # b_fc cancels inside BatchNorm
    hn  = relu((h - mu) * rsqrt(var + eps) * gamma + beta)   # batch stats over ALL B rows
    out = hn @ W_clf + b_clf
    out[i, j] = -100 where mask2[cate2[i], j]

Distribution: pure data-parallel over the batch (2048 rows/core on 8 cores);
weights/mask table replicated. BN batch statistics: ONE ncfw AllGather of
per-core [sum_h, sum_h^2] (bf16 [128,8]) + local tree-fold.

Design notes (trace-driven):
  - fc matmul in fp8 e4m3 DoubleRow perf mode (2 contraction rows/cycle).
    W_fc pre-scaled by 64 host-side (e4m3 range); the scale cancels in BN
    (eps scaled by 64^2).
  - x marshaled [KP, 2, 128, BL] so every load is a fully-linear
    2048B-per-partition DMA; 24 transfers rotate over the sync/scalar/gpsimd
    queues kp-ascending. The feed is HBM/descriptor-supply bound (~150GB/s),
    which together with the PE floor puts fc at ~33-35us.
  - fc runs f-grouped passes (f01 with 8 PSUM banks, then f2, f3) so each
    stationary weight pair serves 2048 rhs cols (half the LDWEIGHTS) and
    only f3's four drains gate the stats fold.
  - EXACTLY ONE collective. Measured: the ncfw entry barrier grows ~13us per
    additional cc op (a warmup AllGather is strictly counterproductive), cc
    triggers are gated on the entry barrier, and the first op pays ~11.5us
    cold dispatch. Keep nothing queued behind the trigger on gpsimd.
  - ACT tables for Sqrt/Relu preloaded via dummy activations in the post-fc
    idle window (the lazy table swap was 1.28us on the critical path).
  - BN consts all-vector except the Sqrt (constants folded into stt
    immediates); BN apply on ACT (per-partition scale/bias native) in
    256/256/512/1024-col groups: first clf matmuls start ~1.8us after
    consts while later groups amortize ACT op overhead.
  - clf bias added on DVE in-PSUM (tensor_tensor with a host-replicated
    [128,125] bias tile) instead of 32 extra PE instructions; masking via
    copy_predicated into a -100-memset buffer using an indirect-DMA
    gathered keep-table.
  - output stored partition-major as 4 fully-linear 2KB/partition DMAs
    alternating sync/scalar; host unshuffles.
"""

import numpy as np
import ml_dtypes

B, D, H, C3, C2 = 16384, 2048, 512, 125, 64
NCORES = 8
BL = B // NCORES      # 2048 rows per core
KP = D // 256         # 8 fp8 DoubleRow contraction pairs (2x128 each)
RF = H // 128         # 4 feature chunks
NRC = BL // 512       # 4 row chunks of 512
NT = BL // 128        # 16 output row tiles of 128
BN_EPS = 1e-5
MASK_VAL = -100.0
WSCALE = 64.0         # host pre-scale on W_fc; cancels in BN (eps scaled too)

_CACHE = {}


def _build_nc():
    from contextlib import ExitStack

    import concourse.bass as bass
    import concourse.tile as tile
    from concourse import bacc, mybir

    f32 = mybir.dt.float32
    bf16 = mybir.dt.bfloat16
    fp8 = mybir.dt.float8e4
    i32 = mybir.dt.int32
    AF = mybir.ActivationFunctionType
    OP = mybir.AluOpType
    DR = mybir.MatmulPerfMode.DoubleRow

    nc = bacc.Bacc("TRN2", target_bir_lowering=False, debug=False, num_devices=NCORES)

    xq_d = nc.dram_tensor("xq", [KP, 2, 128, BL], fp8, kind="ExternalInput")
    wfc_d = nc.dram_tensor("wfc", [128, KP * RF * 2 * 128], fp8, kind="ExternalInput")
    wclf_d = nc.dram_tensor("wclf", [128, RF * C3], bf16, kind="ExternalInput")
    bclf_d = nc.dram_tensor("bclf", [128, C3], f32, kind="ExternalInput")
    m2_d = nc.dram_tensor("m2", [C2, C3], mybir.dt.uint8, kind="ExternalInput")
    cat_d = nc.dram_tensor("cat", [128, NT], i32, kind="ExternalInput")
    # partition-major output: out[p, t*C3+c] = row t*128+p; host unshuffles.
    out_d = nc.dram_tensor("out", [128, NT * C3], f32, kind="ExternalOutput")

    with tile.TileContext(nc) as tc, ExitStack() as ctx:
        xpool = ctx.enter_context(tc.tile_pool(name="xq", bufs=KP))
        wpool = ctx.enter_context(tc.tile_pool(name="w", bufs=1))
        hpool = ctx.enter_context(tc.tile_pool(name="h", bufs=RF))
        hnpool = ctx.enter_context(tc.tile_pool(name="hn", bufs=RF))
        hsqpool = ctx.enter_context(tc.tile_pool(name="hsq", bufs=2))
        smallpool = ctx.enter_context(tc.tile_pool(name="small", bufs=1))
        psum_fc = ctx.enter_context(tc.tile_pool(name="psfc", bufs=8, space="PSUM"))
        drampool = ctx.enter_context(tc.tile_pool(name="dram", bufs=1, space="DRAM"))

        # ---- loads: fully-linear DMAs rotated over 3 queues, kp-ascending;
        # small late-use tensors (cate/bclf/wclf) load after the x stream.
        wfc_sb = wpool.tile([128, KP * RF * 2 * 128], fp8, tag="wfc")
        xts = [xpool.tile([128, 2 * BL], fp8, tag="xq", name=f"xq{k}") for k in range(KP)]

        qs = [nc.sync, nc.scalar, nc.gpsimd]
        WCH = RF * 2 * 128  # 1024 wfc cols per pair
        for kp in range(KP):
            qs[kp % 3].dma_start(
                wfc_sb[:, kp * WCH : (kp + 1) * WCH],
                wfc_d.ap()[:, kp * WCH : (kp + 1) * WCH],
            )
            for j in range(2):
                qs[(kp + 1 + j) % 3].dma_start(
                    xts[kp][:, j * BL : (j + 1) * BL],
                    xq_d.ap()[kp, j],
                )

        cate_sb = smallpool.tile([128, NT], i32, tag="cate")
        nc.sync.dma_start(cate_sb[:], cat_d.ap())
        bclf_sb = smallpool.tile([128, C3], f32, tag="bclf")
        nc.sync.dma_start(bclf_sb[:], bclf_d.ap())
        wclf_sb = wpool.tile([128, RF * C3], bf16, tag="wclf")
        nc.sync.dma_start(wclf_sb[:], wclf_d.ap())

        def x3(ap):  # [128, 2*BL] -> [128, 2, BL]
            return ap.rearrange("p (j r) -> p j r", j=2)

        eps_sb = smallpool.tile([128, 1], f32, tag="eps")
        nc.vector.memset(eps_sb[:], BN_EPS * WSCALE * WSCALE)

        # ---- fc matmul (fp8 DoubleRow) + streaming BN stats ----
        h_sb = [hpool.tile([128, BL], bf16, tag="h", name=f"h{f}") for f in range(RF)]
        # stats col layout: r*8 + f = sum_h, r*8 + 4 + f = sum_h^2
        sums_sb = smallpool.tile([128, NRC * 8], f32, tag="sums")

        def wslice(kp, f):  # [128, 2, 128] stationary pair
            off = ((kp * RF + f) * 2) * 128
            return wfc_sb[:, off : off + 256].rearrange("p (j m) -> p j m", j=2)

        def drain(f, r, psum):
            nc.vector.tensor_scalar(
                out=h_sb[f][:, r * 512 : (r + 1) * 512],
                in0=psum[:],
                scalar1=1.0,
                scalar2=None,
                op0=OP.mult,
                op1=OP.add,
                accum_out=sums_sb[:, r * 8 + f : r * 8 + f + 1],
            )
            hsq = hsqpool.tile([128, 512], bf16, tag="hsq", name=f"hsq{f}_{r}")
            nc.scalar.activation(
                out=hsq[:],
                in_=psum[:],
                func=AF.Square,
                accum_out=sums_sb[:, r * 8 + 4 + f : r * 8 + 4 + f + 1],
            )

        # pass A: f-pair (0,1), all 4 row chunks per stationary weight pair
        # (2048 rhs cols per LDWEIGHTS); then f=2 and f=3 as separate
        # sub-passes so only f3's four drains gate the stats fold at the end.
        def fc_pass(fs):
            psums = {
                (f, r): psum_fc.tile([128, 512], f32, tag="ps", name=f"ps{f}_{r}")
                for f in fs
                for r in range(4)
            }
            for kp in range(KP):
                for f in fs:
                    lhsT = wslice(kp, f)
                    for r in range(4):
                        nc.tensor.matmul(
                            psums[(f, r)][:],
                            lhsT=lhsT,
                            rhs=x3(xts[kp][:])[:, :, r * 512 : (r + 1) * 512],
                            start=(kp == 0),
                            stop=(kp == KP - 1),
                            perf_mode=DR,
                        )
            for f in fs:
                for r in range(4):
                    drain(f, r, psums[(f, r)])

        fc_pass((0, 1))
        fc_pass((2,))
        fc_pass((3,))

        # ACT table preload: Sqrt/Relu live in a different table than Square;
        # force the swap now (ACT idle, AG in flight) instead of paying
        # ~1.3us on the post-AG critical path. Same queue as the Squares ->
        # runs right after the last drain.
        tbl_sb = smallpool.tile([128, 1], f32, tag="tbl")
        nc.scalar.activation(out=tbl_sb[:], in_=eps_sb[:], func=AF.Sqrt)
        nc.scalar.activation(out=tbl_sb[:], in_=eps_sb[:], func=AF.Relu)

        # fold the row-chunk partials (vector queue, right before the
        # collective trigger below)
        t01 = smallpool.tile([128, 8], f32, tag="t01")
        nc.vector.tensor_tensor(out=t01[:], in0=sums_sb[:, 0:8], in1=sums_sb[:, 8:16], op=OP.add)
        t23 = smallpool.tile([128, 8], f32, tag="t23")
        nc.vector.tensor_tensor(out=t23[:], in0=sums_sb[:, 16:24], in1=sums_sb[:, 24:32], op=OP.add)
        stats = smallpool.tile([128, 8], bf16, tag="stats")
        nc.vector.tensor_tensor(out=stats[:], in0=t01[:], in1=t23[:], op=OP.add)

        # ---- stats AllReduce across the 8 cores ----
        # (vs AllGather+local fold: the op may be a bit slower, but the
        # readback is a tiny linear [128,8] DMA and the 3-op fold tree
        # disappears from the post-collective critical path)
        cc_in = drampool.tile([128, 8], bf16, tag="ccin")
        cc_out = drampool.tile([128, 8], bf16, tag="ccout", addr_space="Shared")
        ccdma = nc.sync.dma_start(cc_in[:], stats[:])
        nc.gpsimd.collective_compute(
            "AllReduce",
            OP.add,
            replica_groups=[list(range(NCORES))],
            ins=[cc_in[:].opt()],
            outs=[cc_out[:].opt()],
        )
        # masked positions default to -100; gathers fill the keep-table
        outs_sb = smallpool.tile([128, NT * C3], f32, tag="outs")
        nc.gpsimd.memset(outs_sb[:], MASK_VAL)
        mask_sb = smallpool.tile([128, NT * C3], mybir.dt.uint8, tag="mask")
        for t in range(NT):
            nc.gpsimd.indirect_dma_start(
                out=mask_sb[:, t * C3 : (t + 1) * C3],
                out_offset=None,
                in_=m2_d.ap(),
                in_offset=bass.IndirectOffsetOnAxis(ap=cate_sb[:, t : t + 1], axis=0),
            )

        stats_all = smallpool.tile([128, 8], bf16, tag="statsall")
        nc.gpsimd.dma_start(stats_all[:], cc_out[:])

        # PE warm-up during the AG wait: HAM throttles an idle PE; dummy
        # matmuls (gated on the local stats DMA, NOT the collective) keep
        # duty up for the clf matmuls.
        warm_ps = psum_fc.tile([128, 512], f32, tag="ps", name="warmps")
        for wi in range(16):
            mi = nc.tensor.matmul(
                warm_ps[:],
                lhsT=h_sb[0][:, 0:128],
                rhs=h_sb[0][:, 0:512],
                start=True,
                stop=True,
                skip_group_check=True,
            )
            if wi == 0:
                tile.add_dep_helper(mi.ins, ccdma.ins, sync=True, reason="warm PE during AG")

        # ---- BN consts: svec = rsqrt(var'+eps'), tvec = -mu'*svec ----
        # (scaled domain: stats are of h' = 64*h, eps' = eps*64^2; gamma==1 /
        # beta==0 per setup_inputs — asserted host-side. All-vector except the
        # Sqrt; constants folded into stt immediates.)
        S1 = stats_all[:, 0:RF]
        S2 = stats_all[:, RF : 2 * RF]
        m2s = smallpool.tile([128, RF], f32, tag="m2s")
        nc.vector.scalar_tensor_tensor(
            out=m2s[:], in0=S1, scalar=1.0 / (B * B), in1=S1, op0=OP.mult, op1=OP.mult,
        )
        var = smallpool.tile([128, RF], f32, tag="var")
        nc.vector.scalar_tensor_tensor(
            out=var[:], in0=S2, scalar=1.0 / B, in1=m2s[:], op0=OP.mult, op1=OP.subtract,
        )
        std = smallpool.tile([128, RF], f32, tag="std")
        nc.scalar.activation(std[:], var[:], AF.Sqrt, bias=eps_sb[:, 0:1])
        svec = smallpool.tile([128, RF], f32, tag="svec")
        nc.vector.reciprocal(svec[:], std[:])
        tvec = smallpool.tile([128, RF], f32, tag="tvec")
        nc.vector.scalar_tensor_tensor(
            out=tvec[:], in0=S1, scalar=-1.0 / B, in1=svec[:], op0=OP.mult, op1=OP.mult,
        )

        # ---- BN apply + relu, then clf matmul + bias + mask + store ----
        # BN apply on ACT (per-partition scale/bias native there) in 512-col
        # chunks: the first 4-tile clf group starts ~2.6us after consts.
        # Bias is added on DVE (broadcast tensor_tensor) — saves 32 PE
        # instructions vs the bias-matmul trick.
        hn_sb = [hnpool.tile([128, BL], bf16, tag="hn", name=f"hn{f}") for f in range(RF)]
        groups = [(0, 256), (256, 512), (512, 1024), (1024, 2048)]
        for gi, (c0, c1) in enumerate(groups):
            for f in range(RF):
                nc.scalar.activation(
                    out=hn_sb[f][:, c0:c1],
                    in_=h_sb[f][:, c0:c1],
                    func=AF.Relu,
                    scale=svec[:, f : f + 1],
                    bias=tvec[:, f : f + 1],
                )
            for t in range(c0 // 128, c1 // 128):
                po = psum_fc.tile([128, C3], f32, tag="ps", name=f"po{t}")
                for f in range(RF):
                    nc.tensor.matmul(
                        po[:],
                        lhsT=hn_sb[f][:, t * 128 : (t + 1) * 128],
                        rhs=wclf_sb[:, f * C3 : (f + 1) * C3],
                        start=(f == 0),
                        stop=(f == RF - 1),
                    )
                nc.vector.tensor_tensor(out=po[:], in0=po[:], in1=bclf_sb[:], op=OP.add)
                nc.vector.copy_predicated(
                    outs_sb[:, t * C3 : (t + 1) * C3],
                    mask_sb[:, t * C3 : (t + 1) * C3],
                    po[:],
                )
                # one fully-linear store per 4-tile group (2KB/partition)
                if t % 4 == 3:
                    t0 = t - 3
                    gs = slice(t0 * C3, (t0 + 4) * C3)
                    eng = nc.sync if (t0 // 4) % 2 == 0 else nc.scalar
                    eng.dma_start(out_d.ap()[:, gs], outs_sb[:, gs])

    nc.compile()
    return nc


def _get_nc():
    if "nc" not in _CACHE:
        _CACHE["nc"] = _build_nc()
    return _CACHE["nc"]


def make_in_maps(**inputs):
    """Host-side marshaling: shard/cast/layout the full inputs per core."""
    bf16 = ml_dtypes.bfloat16
    e4m3 = ml_dtypes.float8_e4m3  # IEEE variant, max 240 — matches TRN FP8_EXP4

    x = np.asarray(inputs["swem_vec"], dtype=np.float32)
    xT8 = np.ascontiguousarray(x.T).astype(e4m3)  # [D, B]

    wfc = np.asarray(inputs["W_fc"], dtype=np.float32) * WSCALE
    wfc8 = np.ascontiguousarray(
        wfc.reshape(KP, 2, 128, RF, 128).transpose(2, 0, 3, 1, 4).reshape(128, -1)
    ).astype(e4m3)

    wclf = np.asarray(inputs["W_clf"], dtype=np.float32)
    wclf_h = np.ascontiguousarray(
        wclf.reshape(RF, 128, C3).transpose(1, 0, 2).reshape(128, RF * C3)
    ).astype(bf16)
    bclf = np.ascontiguousarray(np.broadcast_to(np.asarray(inputs["b_clf"], dtype=np.float32)[None, :], (128, C3)))
    # the device program specializes gamma==1 / beta==0 (reference
    # setup_inputs hardcodes them); fail loudly if that ever changes
    assert np.all(np.asarray(inputs["gamma"], dtype=np.float32) == 1.0), "gamma != 1"
    assert np.all(np.asarray(inputs["beta"], dtype=np.float32) == 0.0), "beta != 0"
    m2 = (~np.asarray(inputs["mask2"])).astype(np.uint8)  # 1 = keep, 0 = mask to -100
    cate = np.asarray(inputs["cate2"]).astype(np.int32)

    in_maps = []
    for c in range(NCORES):
        sl = slice(c * BL, (c + 1) * BL)
        xc = xT8[:, sl]  # [D, BL]
        # [KP, 2, 128, BL]: d = kp*256 + j*128 + p — each (kp, j) block is a
        # fully-linear 256KB DMA
        xq = np.ascontiguousarray(xc.reshape(KP, 2, 128, BL))
        in_maps.append(
            {
                "xq": xq,
                "wfc": wfc8,
                "wclf": wclf_h,
                "bclf": bclf,
                "m2": m2,
                "cat": np.ascontiguousarray(cate[sl].reshape(NT, 128).T),
            }
        )
    return in_maps


def run(in_maps, trace=False, **kwargs):
    from concourse.bass_utils import run_bass_kernel_spmd

    nc = _get_nc()
    return run_bass_kernel_spmd(
        nc, in_maps, core_ids=list(range(NCORES)), trace=trace, **kwargs
    )


def unshard(res) -> np.ndarray:
    # device output is partition-major [128, NT*C3]; unshuffle to [BL, C3]
    return np.concatenate(
        [
            res.results[c]["out"].reshape(128, NT, C3).transpose(1, 0, 2).reshape(BL, C3)
            for c in range(NCORES)
        ],
        axis=0,
    )


def kernel(**inputs) -> np.ndarray:
    in_maps = make_in_maps(**inputs)
    return unshard(run(in_maps, trace=False))
